# revision 34
# baseline (speedup 1.0000x reference)
"""Trainium2 kernel for nn_Circuit_41936060678727.

The reference is a 10-qubit real-amplitude circuit (CNOT ladders + RY
rotations) applied to an amplitude-embedded batch, measured with PauliZ on
each of the 10 wires.  Every gate is linear in the state, so the whole
8-layer circuit collapses to one fixed 1024x1024 orthogonal matrix M that
depends only on `params` (8x10).  With x padded to 1024 and L2-normalized:

    out[b, p] = sum_z (x[b] @ M[:784, :])[z]^2 * sign_p(z) / ||x[b]||^2

M is orthogonal, so ||x @ M[:784,:]|| = ||x||: the norm falls out of an extra
all-ones column in the sign matrix, and the global scale of y cancels in the
ratio -- which lets the whole first matmul run in scaled fp8.

fp8 scheme (e4m3 = TRN float8e4, max +-240):
  x is uniform[0,1): send xq = e4m3((x - 0.5) * 256); the DC part
  0.5 * (1^T W) is restored exactly by 3 extra contraction rows whose
  stationary values hold a 3-way e4m3 split of the column sums.
  W is sent as e4m3(W * 128) plus, for the rows with the largest residual
  energy, a second "lo" row carrying e4m3(W * 128 - Wh); every spare slot in
  the padded contraction stack carries one of these corrections.
  The stack is padded to NCH * 256 rows and fed to the PE as DoubleRow
  (double-pumped fp8) matmuls: 256-deep contraction per instruction.

Device work per core (batch 16384 data-parallel over 8 cores, 2048 each):
    Y   [1024, 2048] = Wstack^T @ Xstack   (tensor, fp8 DoubleRow, NCH chunks)
    sq  = bf16(Y^2)                        (scalar + vector engines)
    oT  [11, 2048]  = Zaug^T @ sq          (tensor, bf16, K=1024)
Host: out = (oT[:10] / oT[10]) ^T, concat cores.
"""

import numpy as np
import ml_dtypes

E4M3 = ml_dtypes.float8_e4m3     # TRN float8e4: bias 7, max normal 240
BF16 = ml_dtypes.bfloat16

N_QUBITS = 10
DIM = 1 << N_QUBITS          # 1024
N_OUT = 10
D_IN = 784
B_TOTAL = 16384
N_CORES = 8
B_CORE = B_TOTAL // N_CORES  # 2048
GROUP = 512                  # batch columns per matmul (one PSUM bank, fp32)
# batch-column blocks: 512-wide for the pipelined bulk, two 256-wide blocks
# at the end so the serial drain (squares -> mm2 -> copy -> DMA) is short
GW = (512, 512, 512, 256, 256)
GOFF = (0, 512, 1024, 1536, 1792)
N_GROUPS = len(GW)
NCH = 4                      # DoubleRow chunks: NCH*256 contraction slots
NST = NCH * 256              # stack rows (784 hi + 3 DC + lo corrections)
N_DC = 3
Z_CHUNK = 128
N_ZCH = DIM // Z_CHUNK       # 8
ZCOLS = 16                   # cols 0..9 = PauliZ signs, 10 = ones, 11..15 pad
SW = 128.0                   # W scale into fp8 range
SX = 256.0                   # (x - 0.5) scale into fp8 range
MDC = 128.0                  # moving value for the DC rows
N_SQ_SCALAR = 4              # z-chunks squared on scalar engine (rest: vector)


# ----------------------------------------------------------------------------
# Host-side precompute: collapse the circuit to W = M[:784, :]
# ----------------------------------------------------------------------------

def _apply_ry(S, theta, q):
    B = S.shape[0]
    left, right = 1 << q, 1 << (N_QUBITS - q - 1)
    s = S.reshape(B, left, 2, right)
    c, sn = np.cos(theta / 2), np.sin(theta / 2)
    s0 = c * s[:, :, 0] - sn * s[:, :, 1]
    s1 = sn * s[:, :, 0] + c * s[:, :, 1]
    return np.stack([s0, s1], axis=2).reshape(B, DIM)


def _apply_cnot(S, q):
    B = S.shape[0]
    left, right = 1 << q, 1 << (N_QUBITS - q - 2)
    s = S.reshape(B, left, 2, 2, right)
    s = np.concatenate([s[:, :, :1], np.flip(s[:, :, 1:], axis=3)], axis=2)
    return s.reshape(B, DIM)


def _build_W(params):
    """Circuit applied to basis rows e_0..e_783 -> W[784, 1024], fp64."""
    w = np.pi * np.tanh(params.astype(np.float64))
    S = np.zeros((D_IN, DIM), dtype=np.float64)
    S[np.arange(D_IN), np.arange(D_IN)] = 1.0
    for l in range(params.shape[0]):
        for start in (0, 1):
            for i in range(start, N_QUBITS - 1, 2):
                S = _apply_cnot(S, i)
        for i in range(N_QUBITS):
            S = _apply_ry(S, w[l, i], i)
    return S


def _q8(a):
    return np.clip(a, -240.0, 240.0).astype(E4M3).astype(np.float32)


def _build_wt(params):
    """Stationary payload [128, NCH*2*1024] e4m3 + the stack row->x map."""
    W64 = _build_W(params)
    W = W64.astype(np.float32)

    Wh = _q8(W * SW)
    Wl = _q8(W * SW - Wh)                       # residual, scaled units
    u = 0.5 * W64.sum(axis=0)                   # DC: 0.5 * (1^T W)
    T = (u * SW * SX / MDC).astype(np.float32)  # stationary DC total
    t1 = _q8(T)
    t2 = _q8(T - t1)
    t3 = _q8(T - t1 - t2)

    stack = np.zeros((NST, DIM), dtype=np.float32)
    xsrc = np.full(NST, -1, dtype=np.int64)     # -1 zero, -2 DC const
    stack[0:D_IN] = Wh
    xsrc[0:D_IN] = np.arange(D_IN)
    stack[D_IN + 0] = t1
    stack[D_IN + 1] = t2
    stack[D_IN + 2] = t3
    xsrc[D_IN:D_IN + N_DC] = -2
    nlo = min(NST - D_IN - N_DC, D_IN)
    sel = np.argsort(-(Wl ** 2).sum(axis=1))[:nlo]
    stack[D_IN + N_DC:D_IN + N_DC + nlo] = Wl[sel]
    xsrc[D_IN + N_DC:D_IN + N_DC + nlo] = sel

    # [NST, 1024] -> [NCH, 2, 128, 1024] -> [128, NCH, 2, 1024]
    wt = stack.reshape(NCH, 2, 128, DIM).transpose(2, 0, 1, 3)
    wt = np.ascontiguousarray(wt.reshape(128, NCH * 2 * DIM)).astype(E4M3)
    return wt, xsrc


def _build_xt(x, xsrc):
    """Moving payload per core: [128, NCH*2*B_CORE] e4m3, layout [p,g,c,s,512]
    so each batch group is one contiguous DMA."""
    xq = ((x - 0.5) * SX).clip(-240, 240).astype(E4M3)   # [B_TOTAL, 784]
    outs = []
    for c in range(N_CORES):
        xs = np.zeros((NST, B_CORE), dtype=E4M3)
        m = xsrc >= 0
        xs[m] = xq[c * B_CORE:(c + 1) * B_CORE, xsrc[m]].T
        xs[xsrc == -2] = E4M3(MDC)
        # per group block: [NCH, 2, 128, w] -> [128, NCH, 2, w], then concat
        blocks = []
        for off, w in zip(GOFF, GW):
            blk = xs[:, off:off + w].reshape(NCH, 2, 128, w).transpose(2, 0, 1, 3)
            blocks.append(blk.reshape(128, NCH * 2 * w))
        outs.append(np.ascontiguousarray(np.concatenate(blocks, axis=1)))
    return outs


def _build_Z():
    z = np.arange(DIM)
    Z = np.zeros((DIM, ZCOLS), dtype=np.float32)
    for p in range(N_OUT):
        Z[:, p] = 1.0 - 2.0 * ((z >> (N_QUBITS - 1 - p)) & 1)
    Z[:, N_OUT] = 1.0
    # device layout [128, 8*16]: chunk z of rows z*128..z*128+128 at cols z*16..
    Zd = Z.reshape(N_ZCH, Z_CHUNK, ZCOLS).transpose(1, 0, 2).reshape(Z_CHUNK, -1)
    return np.ascontiguousarray(Zd).astype(BF16)


# ----------------------------------------------------------------------------
# Bass program (identical SPMD program on all 8 cores)
# ----------------------------------------------------------------------------

_NC_CACHE = {}
TRACE = False           # test harness can flip this for profiling
LAST_RESULTS = None


def _build_bass():
    from contextlib import ExitStack

    import concourse.tile as tile
    from concourse import bacc, mybir

    f32 = mybir.dt.float32
    fp8 = mybir.dt.float8e4
    bf16 = mybir.dt.bfloat16
    DR = mybir.MatmulPerfMode.DoubleRow

    nc = bacc.Bacc(
        "TRN2", target_bir_lowering=False, debug=False, num_devices=N_CORES
    )
    xt_d = nc.declare_dram_parameter("xt", [128, NCH * 2 * B_CORE], fp8,
                                     isOutput=False)
    wt_d = nc.declare_dram_parameter("wt", [128, NCH * 2 * DIM], fp8,
                                     isOutput=False)
    zt_d = nc.declare_dram_parameter("zt", [Z_CHUNK, N_ZCH * ZCOLS], bf16,
                                     isOutput=False)
    out_d = nc.declare_dram_parameter("out", [N_OUT + 1, B_CORE], f32,
                                      isOutput=True)

    with ExitStack() as ctx:
        tc = ctx.enter_context(tile.TileContext(nc))
        wpool = ctx.enter_context(tc.tile_pool(name="w", bufs=1))
        zpool = ctx.enter_context(tc.tile_pool(name="z", bufs=1))
        xpool = ctx.enter_context(tc.tile_pool(name="x", bufs=1))
        sqpool = ctx.enter_context(tc.tile_pool(name="sq", bufs=1))
        opool = ctx.enter_context(tc.tile_pool(name="osb", bufs=1))
        # 7 py banks + 1 po bank = all 8 PSUM banks
        pypool = ctx.enter_context(tc.tile_pool(name="py", bufs=7, space="PSUM"))
        popool = ctx.enter_context(tc.tile_pool(name="po", bufs=1, space="PSUM"))

        # PE pre-warm: full-array (K=128) bf16 matmuls bridge the DMA prefix
        # AND release the HAM clock-gate (1.2 -> 2.4 GHz) -- the activity
        # monitor watches array occupancy, so K=1 warmups don't count.
        warm_in = opool.tile([128, 384], bf16, name="warm_in")
        nc.any.memset(warm_in[:], 0.0)
        warm_ps = popool.tile([128, 256], f32, name="warm_ps", tag="po")
        for i in range(17):
            nc.tensor.matmul(
                warm_ps[:],
                lhsT=warm_in[:, 0:128],
                rhs=warm_in[:, 128:384],
                start=True,
                stop=True,
                skip_group_check=True,
            )

        # Input DMAs on ONE HWDGE ring (sync), in consumption order:
        # (w_c, x_g0_c) pairs, the sign matrix, then one DMA per later group.
        # x payload layout is [p, g, c, s, w] with per-group widths GW.
        PB = [NCH * 2 * off for off in GOFF]   # payload base per group
        w_sb = [wpool.tile([128, 2, DIM], fp8, tag=f"w{c}", name=f"w{c}")
                for c in range(NCH)]
        z_sb = zpool.tile([Z_CHUNK, N_ZCH * ZCOLS], bf16)
        x_sb = [xpool.tile([128, 2 * NCH, GW[g]], fp8, tag=f"xg{g}",
                           name=f"xg{g}")
                for g in range(N_GROUPS)]
        for c in range(NCH):
            nc.sync.dma_start(w_sb[c][:],
                              wt_d[:, c * 2 * DIM:(c + 1) * 2 * DIM])
            nc.sync.dma_start(
                x_sb[0][:, 2 * c:2 * c + 2, :],
                xt_d[:, c * 2 * GW[0]:(c + 1) * 2 * GW[0]],
            )
            if c == 0:
                nc.sync.dma_start(z_sb[:], zt_d[:, :])
        for g in range(1, N_GROUPS):
            nc.sync.dma_start(
                x_sb[g][:], xt_d[:, PB[g]:PB[g] + NCH * 2 * GW[g]]
            )

        out_sb = opool.tile([N_OUT + 1, B_CORE], f32)

        for g in range(N_GROUPS):
            off, gw = GOFF[g], GW[g]
            # c outer / z inner: each landed (w_c, x_c_g) pair unlocks 8
            # matmuls (7 py banks; the 8th z waits for a square to free one)
            pys = [
                pypool.tile([Z_CHUNK, GROUP], f32, tag="py", name=f"py_{g}_{z}")
                for z in range(N_ZCH)
            ]
            for c in range(NCH):
                for z in range(N_ZCH):
                    nc.tensor.matmul(
                        pys[z][:, 0:gw],
                        lhsT=w_sb[c][:, :, z * Z_CHUNK:(z + 1) * Z_CHUNK],
                        rhs=x_sb[g][:, 2 * c:2 * c + 2, :],
                        start=(c == 0),
                        stop=(c == NCH - 1),
                        perf_mode=DR,
                        skip_group_check=True,
                    )
            po = popool.tile([N_OUT + 1, GROUP], f32)
            last = g == N_GROUPS - 1
            sqs = []
            for z in range(N_ZCH):
                sq = sqpool.tile([Z_CHUNK, GROUP], bf16, tag=f"sq{z}",
                                 name=f"sq_{g}_{z}")
                if last and z in (1, 4, 7):
                    # final drain: split squares across scalar + vector so
                    # neither engine serializes the tail (DVE can't read PSUM
                    # twice, so copy then multiply)
                    tmp = sqpool.tile([Z_CHUNK, GROUP], f32, tag=f"tmp{z}",
                                      name=f"tmp_{g}_{z}")
                    nc.vector.tensor_copy(tmp[:, 0:gw], pys[z][:, 0:gw])
                    nc.vector.tensor_mul(sq[:, 0:gw], tmp[:, 0:gw],
                                         tmp[:, 0:gw])
                else:
                    nc.scalar.square(sq[:, 0:gw], pys[z][:, 0:gw])
                sqs.append(sq)
            # mm2 accumulation order: DESCENDING for pipelined groups, so the
            # first mm2 (z=7) only becomes ready once all squares are done and
            # the clump of 8 runs back-to-back (one stationary-switch penalty
            # instead of eight); ASCENDING for the drain group, where mm2
            # should chase the squares as they complete.
            zorder = range(N_ZCH) if last else range(N_ZCH - 1, -1, -1)
            for i, z in enumerate(zorder):
                nc.tensor.matmul(
                    po[:, 0:gw],
                    lhsT=z_sb[:, z * ZCOLS: z * ZCOLS + N_OUT + 1],
                    rhs=sqs[z][:, 0:gw],
                    start=(i == 0),
                    stop=(i == N_ZCH - 1),
                    skip_group_check=True,
                )
            nc.vector.tensor_copy(out_sb[:, off:off + gw], po[:, 0:gw])
            # single_packet: the tiny result DMA rides one ring instead of
            # synchronizing all 16 (the striped completion costs ~1.5us)
            nc.sync.dma_start(
                out_d[:, off:off + gw],
                out_sb[:, off:off + gw],
                single_packet=True,
            )

    nc.finalize()
    return nc


def _get_nc():
    if "nc" not in _NC_CACHE:
        _NC_CACHE["nc"] = _build_bass()
    return _NC_CACHE["nc"]


# ----------------------------------------------------------------------------
# Entry point
# ----------------------------------------------------------------------------

def kernel(input, params):
    global LAST_RESULTS
    from concourse.bass_utils import run_bass_kernel_spmd

    x = np.ascontiguousarray(np.asarray(input, dtype=np.float32))
    p = np.asarray(params, dtype=np.float32)

    wt, xsrc = _build_wt(p)
    xts = _build_xt(x, xsrc)
    Z = _build_Z()

    nc = _get_nc()
    in_maps = [{"xt": xts[c], "wt": wt, "zt": Z} for c in range(N_CORES)]

    res = run_bass_kernel_spmd(nc, in_maps, list(range(N_CORES)), trace=TRACE)
    LAST_RESULTS = res

    outs = []
    for c in range(N_CORES):
        o = res.results[c]["out"]                 # [11, 2048]
        outs.append((o[:N_OUT] / o[N_OUT:N_OUT + 1]).T)
    return np.ascontiguousarray(np.concatenate(outs, axis=0).astype(np.float32))
